# revision 1
# baseline (speedup 1.0000x reference)
"""Trainium2 Bass kernel for nn_CA1AttentionGate.

Computes, for full inputs (B=1, S=8192, H=1024, F=128, K=2):
    temporal = relu(t @ Wt1 + bt1) @ Wt2 + bt2          [K,F]
    mem      = dg_features + temporal                    [K,F]
    qmean    = query.mean(axis=1)                        [1,H]
    score_k  = tanh([mem_k ; qmean] @ Wa1 + ba1) @ Wa2 + ba2
    w_k      = sigmoid(score_k)
    g_k      = mem_k @ Wg + bg                           [K,H]
    row[s]   = (1/K) * sum_k w_k * (g_k . key[s])        [S]
    out      = broadcast(row) -> [1,1,S,S]

Sharding: sequence-parallel over the key/seq axis across 8 cores.  Each
core computes the final gate row for its 1024 key positions and writes
its dense [8192, 1024] column slab of the output.  The only cross-core
quantity is qmean: each core reduces its query shard and a 4KB AllReduce
completes the mean (fallback variant replicates the full query read).
"""

import os

import numpy as np

SEQ = 8192
H = 1024
F = 128
K = 2
NCORES = 8
SHARD = SEQ // NCORES  # 1024
NT = SHARD // 128  # 8 key tiles per shard

_PROG_CACHE = {}


def _build(use_collective: bool):
    import concourse.bacc as bacc
    import concourse.bass as bass
    import concourse.tile as tile
    from concourse import mybir
    from concourse.tile_rust import add_dep_helper

    AF = mybir.ActivationFunctionType
    ALU = mybir.AluOpType
    f32 = mybir.dt.float32

    nc = bacc.Bacc(
        "TRN2",
        target_bir_lowering=False,
        debug=False,
        num_devices=NCORES,
    )

    def din(name, shape):
        return nc.dram_tensor(name, list(shape), f32, kind="ExternalInput").ap()

    q_rows = SHARD if use_collective else SEQ
    qs = din("qs", (q_rows, H))
    ks = din("ks", (SHARD, H))
    dg = din("dg", (K, F))
    ts = din("ts", (K,))
    Wt1 = din("Wt1", (1, F // 4))
    bt1 = din("bt1", (F // 4,))
    Wt2 = din("Wt2", (F // 4, F))
    bt2 = din("bt2", (F,))
    Wa1 = din("Wa1", (F + H, F))
    ba1 = din("ba1", (F,))
    Wa2 = din("Wa2", (F, 1))
    ba2 = din("ba2", (1,))
    Wg = din("Wg", (F, H))
    bg = din("bg", (H,))
    # column of 1/SEQ: the qsum partition-reduce matmul yields the scaled
    # mean contribution directly
    scale_col = din("scale_col", (128, 1))
    out = nc.dram_tensor("out", [SEQ, SHARD], f32, kind="ExternalOutput").ap()

    def bcast(ap, n):
        # replicate a DRAM row across n partitions (stride-0 partition dim)
        return bass.AP(tensor=ap.tensor, offset=ap.offset, ap=[[0, n]] + list(ap.ap))

    def col(ap, n):
        # load a flat [n] DRAM vector as an [n, 1] column
        return bass.AP(tensor=ap.tensor, offset=ap.offset, ap=[[1, n], [n, 1]])

    with tile.TileContext(nc) as tc:
        with (
            tc.tile_pool(name="consts", bufs=1) as cp,
            tc.tile_pool(name="work", bufs=1) as wp,
            tc.tile_pool(name="qstream", bufs=8) as qp,
            tc.tile_pool(name="scratch", bufs=3) as sp,
            tc.tile_pool(name="psum_small", bufs=2, space="PSUM") as pps,
            tc.tile_pool(name="psum_big", bufs=3, space="PSUM") as ppb,
            tc.tile_pool(name="dram", bufs=1, space="DRAM") as dp,
        ):
            # ---- constant / weight loads (sync DGE ring) ---------------
            sc_c = cp.tile([128, 1], f32)
            nc.sync.dma_start(sc_c, scale_col)
            # ---- query shard DMAs get the wire first -------------------
            # (every weight/const below has >=10us of slack; the query
            # stream feeds the collective and must not queue behind them)
            nq = q_rows // 128
            qv = qs.rearrange("(t p) h -> t p h", p=128)
            qtiles = []
            q_insts = []
            for i in range(nq):
                qt = qp.tile([128, H], f32, tag="qt")
                q_insts.append(nc.sync.dma_start(qt, qv[i]))
                qtiles.append(qt)

            Wt2_sb = cp.tile([F // 4, F], f32)
            nc.sync.dma_start(Wt2_sb, Wt2)
            Wa1m_sb = cp.tile([128, 128], f32)
            nc.sync.dma_start(Wa1m_sb, Wa1[0:F, :])
            # qmean rows of Wa1 re-paired to the interleaved qmT layout:
            # chunk c pairs with rows {128 + i*8 + c}
            Wa1q_sb = cp.tile([128, 8, 128], f32)
            nc.sync.dma_start(
                Wa1q_sb, Wa1[F : F + H, :].rearrange("(i c) f -> i c f", c=8)
            )
            Wa2_sb = cp.tile([F, 1], f32)
            nc.sync.dma_start(Wa2_sb, Wa2)
            Wg_sb = cp.tile([F, H], f32)
            nc.sync.dma_start(Wg_sb, Wg)
            dgT_sb = cp.tile([F, K], f32)
            nc.sync.dma_start(dgT_sb, dg.rearrange("k f -> f k"))
            tb_sb = cp.tile([F // 4, K], f32)
            nc.sync.dma_start(tb_sb, bcast(ts, F // 4))
            Wt1T_sb = cp.tile([F // 4, 1], f32)
            nc.sync.dma_start(Wt1T_sb, col(Wt1, F // 4))
            bt1T_sb = cp.tile([F // 4, 1], f32)
            nc.sync.dma_start(bt1T_sb, col(bt1, F // 4))
            bt2T_sb = cp.tile([F, 1], f32)
            nc.sync.dma_start(bt2T_sb, col(bt2, F))
            ba1T_sb = cp.tile([F, 1], f32)
            nc.sync.dma_start(ba1T_sb, col(ba1, F))
            ba2b_sb = cp.tile([1, 1], f32)
            nc.sync.dma_start(ba2b_sb, bcast(ba2, 1))
            bg_sb = cp.tile([1, H], f32)
            nc.sync.dma_start(bg_sb, bg.rearrange("(a h) -> a h", a=1))

            # warm the ACT function tables used late in the critical path
            warm1 = cp.tile([1, 1], f32)
            nc.scalar.activation(warm1, sc_c[0:1, :], AF.Tanh)
            warm2 = cp.tile([1, 1], f32)
            nc.scalar.activation(warm2, sc_c[0:1, :], AF.Sigmoid)
            # key shard: interleaved, ktiles[j][p, :] = ks[p*NT + j, :];
            # explicitly ordered after the query stream so the query mean
            # (-> collective) is not starved of read bandwidth
            kv = ks.rearrange("(p t) h -> p t h", t=NT)
            ktiles = []
            for j in range(NT):
                kt = cp.tile([128, H], f32, tag=f"ks{j}")
                ki = nc.sync.dma_start(kt, kv[:, j, :])
                add_dep_helper(ki.ins, q_insts[-1].ins,
                               reason="key reads after query")
                ktiles.append(kt)

            # ---- query accumulate on DVE: head of the critical path ----
            qacc = wp.tile([128, H], f32)
            for i in range(nq):
                if i == 0:
                    nc.vector.tensor_copy(qacc, qtiles[i])
                else:
                    nc.vector.tensor_add(qacc, qacc, qtiles[i])

            # ---- qmean partial (PE first) -> collective ----------------
            qsum_ps = ppb.tile([1, H], f32, tag="big")
            nc.tensor.matmul(
                qsum_ps[:, 0:512], lhsT=sc_c, rhs=qacc[:, 0:512],
                start=True, stop=True,
            )
            nc.tensor.matmul(
                qsum_ps[:, 512:1024], lhsT=sc_c, rhs=qacc[:, 512:1024],
                start=True, stop=True,
            )
            qpart_sb = wp.tile([1, H], f32)
            nc.scalar.copy(qpart_sb, qsum_ps)
            if use_collective:
                cc_in = dp.tile([1, H], f32)
                cc_out = dp.tile([NCORES, H], f32)
                nc.scalar.dma_start(cc_in, qpart_sb)
                nc.gpsimd.collective_compute(
                    "AllGather",
                    ALU.bypass,
                    replica_groups=[list(range(NCORES))],
                    ins=[cc_in.opt()],
                    outs=[cc_out.opt()],
                )
                # park the gather-result load on the (idle) sync ring
                qmTd8 = wp.tile([128, NCORES, 8], f32)
                nc.sync.dma_start(
                    qmTd8, cc_out[:, :].rearrange("d (p c) -> p d c", c=8)
                )

            # ---- temporal MLP -> memT [F, K] ---------------------------
            h1T = wp.tile([F // 4, K], f32)
            nc.vector.tensor_scalar_mul(h1T, tb_sb, Wt1T_sb)
            nc.vector.tensor_scalar_add(h1T, h1T, bt1T_sb)
            nc.vector.tensor_relu(h1T, h1T)
            tT_ps = pps.tile([F, K], f32, tag="small")
            nc.tensor.matmul(tT_ps, lhsT=Wt2_sb, rhs=h1T, start=True, stop=True)
            memT_sb = wp.tile([F, K], f32)
            nc.scalar.activation(memT_sb, tT_ps, AF.Identity, bias=bt2T_sb, scale=1.0)
            nc.vector.tensor_add(memT_sb, memT_sb, dgT_sb)

            # ---- gate rows g_k = mem_k @ Wg + bg  [1, H] ---------------
            def g_row(k):
                g_ps = ppb.tile([1, H], f32, tag="big")
                nc.tensor.matmul(g_ps[:, 0:512], lhsT=memT_sb[:, k : k + 1],
                                 rhs=Wg_sb[:, 0:512], start=True, stop=True)
                nc.tensor.matmul(g_ps[:, 512:1024], lhsT=memT_sb[:, k : k + 1],
                                 rhs=Wg_sb[:, 512:1024], start=True, stop=True)
                return g_ps

            g0_ps = g_row(0)
            g0_sb = wp.tile([1, H], f32, tag="g0r")
            nc.vector.tensor_add(g0_sb, g0_ps, bg_sb)
            gb0 = wp.tile([128, H], f32, tag="gb0")
            nc.gpsimd.partition_broadcast(gb0[:, :], g0_sb[:, :])
            g1_ps = g_row(1)

            # ---- matvec: DVE muls, ACT accumulate-reductions -----------
            # rcc[p, j, k] = sum_h g_k[h] * ks[p*NT+j, h]
            rcc = wp.tile([128, NT, K], f32)

            def matvec(k, gb, js):
                for j in js:
                    prod = sp.tile([128, H], f32, tag="prod")
                    nc.vector.tensor_mul(prod, ktiles[j], gb)
                    junk = sp.tile([128, H], f32, tag="junk")
                    nc.scalar.activation(
                        junk, prod, AF.Copy,
                        accum_out=rcc[:, j, k : k + 1],
                    )

            matvec(0, gb0, range(4))

            # finish g1 mid-stream (its inputs are ready by now)
            g1_sb = wp.tile([1, H], f32, tag="g1r")
            nc.vector.tensor_add(g1_sb, g1_ps, bg_sb)
            gb1 = wp.tile([128, H], f32, tag="gb1")
            nc.gpsimd.partition_broadcast(gb1[:, :], g1_sb[:, :])

            matvec(0, gb0, range(4, NT))
            matvec(1, gb1, range(NT))

            # reshape both anchors at once to an interleaved row:
            # rTi[0, 2*s + k] = r_k[s]   (s = p*NT + j)
            rTi = wp.tile([1, K * SHARD], f32)
            nc.sync.dma_start(rTi[:, :], rcc[:, :, :])

            # ---- post-collective: qmT, scorer, weights -----------------
            # qmT[p, c] = qmean[p*8 + c]  (interleaved reshape layout)
            qmT = wp.tile([128, 8], f32)
            if use_collective:
                # sum gathered partials over d ([p, c, d] view, reduce X)
                nc.vector.tensor_reduce(
                    qmT, qmTd8[:, :, :].rearrange("p d c -> p c d"),
                    axis=mybir.AxisListType.X, op=ALU.add,
                )
            else:
                nc.scalar.dma_start(qmT, qpart_sb[:, :])
            qmTd = wp.tile([128, 8, K], f32)
            nc.vector.tensor_copy(qmTd[:, :, 0:1], qmT[:, :].rearrange("p c -> p c ()"))
            nc.vector.tensor_copy(qmTd[:, :, 1:2], qmT[:, :].rearrange("p c -> p c ()"))
            haT_ps = pps.tile([F, K], f32, tag="small")
            nc.tensor.matmul(haT_ps, lhsT=Wa1m_sb, rhs=memT_sb,
                             start=True, stop=False)
            for c in range(8):
                nc.tensor.matmul(haT_ps, lhsT=Wa1q_sb[:, c, :],
                                 rhs=qmTd[:, c, :], start=False, stop=(c == 7))
            aT_sb = wp.tile([F, K], f32)
            nc.scalar.activation(aT_sb, haT_ps, AF.Tanh, bias=ba1T_sb, scale=1.0)
            scoreT_ps = pps.tile([1, K], f32, tag="small")
            nc.tensor.matmul(scoreT_ps, lhsT=Wa2_sb, rhs=aT_sb, start=True, stop=True)
            wvT_sb = wp.tile([1, K], f32)
            nc.scalar.activation(wvT_sb, scoreT_ps, AF.Sigmoid, bias=ba2b_sb, scale=1.0)
            nc.scalar.mul(wvT_sb, wvT_sb, 1.0 / K)

            # ---- combine anchors in row space, then one broadcast ------
            rt = rTi[:, :]
            r_ev = bass.AP(tensor=rt.tensor, offset=rt.offset,
                           ap=[[K * SHARD, 1], [K, SHARD]])
            r_od = bass.AP(tensor=rt.tensor, offset=rt.offset + 1,
                           ap=[[K * SHARD, 1], [K, SHARD]])
            o_row = wp.tile([1, SHARD], f32)
            o_tmp = wp.tile([1, SHARD], f32)
            nc.vector.tensor_scalar_mul(o_row, r_ev, wvT_sb[0:1, 0:1])
            nc.vector.tensor_scalar_mul(o_tmp, r_od, wvT_sb[0:1, 1:2])
            nc.vector.tensor_add(o_row, o_row, o_tmp)
            out_sb = wp.tile([128, SHARD], f32)
            nc.gpsimd.partition_broadcast(out_sb[:, :], o_row[:, :])

            # ---- output: 64 x [128 rows, SHARD cols], all rows = row ---
            outv = out.rearrange("(b p) n -> b p n", p=128)
            for b in range(SEQ // 128):
                nc.sync.dma_start(outv[b], out_sb)

    nc.compile()
    return nc


def _get_prog(use_collective: bool):
    key = bool(use_collective)
    if key not in _PROG_CACHE:
        _PROG_CACHE[key] = _build(key)
    return _PROG_CACHE[key]


def _make_in_maps(inputs, use_collective: bool):
    q = np.ascontiguousarray(np.asarray(inputs["query"], np.float32)[0])  # [S,H]
    k = np.ascontiguousarray(np.asarray(inputs["key"], np.float32)[0])  # [S,H]
    common = {
        "dg": np.ascontiguousarray(np.asarray(inputs["dg_features"], np.float32)),
        "ts": np.ascontiguousarray(np.asarray(inputs["timestamps"], np.float32)),
        "Wt1": np.ascontiguousarray(np.asarray(inputs["Wt1"], np.float32)),
        "bt1": np.ascontiguousarray(np.asarray(inputs["bt1"], np.float32)),
        "Wt2": np.ascontiguousarray(np.asarray(inputs["Wt2"], np.float32)),
        "bt2": np.ascontiguousarray(np.asarray(inputs["bt2"], np.float32)),
        "Wa1": np.ascontiguousarray(np.asarray(inputs["Wa1"], np.float32)),
        "ba1": np.ascontiguousarray(np.asarray(inputs["ba1"], np.float32)),
        "Wa2": np.ascontiguousarray(np.asarray(inputs["Wa2"], np.float32)),
        "ba2": np.ascontiguousarray(np.asarray(inputs["ba2"], np.float32)),
        "Wg": np.ascontiguousarray(np.asarray(inputs["Wg"], np.float32)),
        "bg": np.ascontiguousarray(np.asarray(inputs["bg"], np.float32)),
        "scale_col": np.full((128, 1), 1.0 / 8192.0, np.float32),
    }
    in_maps = []
    for d in range(NCORES):
        m = dict(common)
        m["ks"] = np.ascontiguousarray(k[d * SHARD : (d + 1) * SHARD])
        if use_collective:
            m["qs"] = np.ascontiguousarray(q[d * SHARD : (d + 1) * SHARD])
        else:
            m["qs"] = q
        in_maps.append(m)
    return in_maps


def _run(inputs, use_collective: bool, trace: bool = False):
    from concourse.bass_utils import run_bass_kernel_spmd

    nc = _get_prog(use_collective)
    in_maps = _make_in_maps(inputs, use_collective)
    res = run_bass_kernel_spmd(
        nc, in_maps, core_ids=list(range(NCORES)), trace=trace
    )
    full = np.empty((1, 1, SEQ, SEQ), np.float32)
    for d in range(NCORES):
        full[0, 0, :, d * SHARD : (d + 1) * SHARD] = res.results[d]["out"]
    return full, res


def kernel(**inputs) -> np.ndarray:
    use_collective = os.environ.get("CA1_NO_COLLECTIVE", "0") != "1"
    try:
        full, _ = _run(inputs, use_collective)
        return full
    except Exception:
        if not use_collective:
            raise
        # fall back to the zero-communication variant (replicated query)
        _PROG_CACHE.pop(True, None)
        full, _ = _run(inputs, False)
        return full



# revision 28
# speedup vs baseline: 1.6912x; 1.6912x over previous
"""Trainium2 Bass kernel for nn_CA1AttentionGate.

Computes, for full inputs (B=1, S=8192, H=1024, F=128, K=2):
    temporal = relu(t @ Wt1 + bt1) @ Wt2 + bt2          [K,F]
    mem      = dg_features + temporal                    [K,F]
    qmean    = query.mean(axis=1)                        [1,H]
    score_k  = tanh([mem_k ; qmean] @ Wa1 + ba1) @ Wa2 + ba2
    w_k      = sigmoid(score_k)
    g_k      = mem_k @ Wg + bg                           [K,H]
    row[s]   = (1/K) * sum_k w_k * (g_k . key[s])        [S]
    out      = broadcast(row) -> [1,1,S,S]

Sharding: sequence-parallel over the key/seq axis across 8 cores.  Each
core computes the final gate row for its 1024 key positions and writes
its dense [8192, 1024] column slab of the output.  The only cross-core
quantity is qmean: each core reduces its query shard on the PE engine
(accumulating PSUM matmuls per arriving tile) and a 4KB AllGather
completes the mean (fallback variant replicates the full query read).

Bandwidth plan (memory-bound problem):
  - query/key stream and the output slab travel as bf16 (host converts;
    error ~5e-3 rel, gate is 2e-2); weights/accumulation stay f32
  - small constants are packed host-side into three dense blocks so the
    SWDGE loads are single-descriptor-run DMAs (column scatter loads
    cost ~3us of Pool-sequencer descriptor generation each)
  - qsum accumulates on PE as query tiles land; the scorer's mem-term
    is precomputed pre-collective; post-collective only the qmean
    matmuls + tanh/score/sigmoid remain
  - final row combine sum_k w_k r_k is one K=2 PE matmul into a
    [128, 1024] PSUM broadcast tile
  - output: 8 big DMAs (stride-0 source re-read, 8 row-blocks each)
    spread across the sync/scalar/gpsimd rings
"""

import os

import numpy as np

SEQ = 8192
H = 1024
F = 128
K = 2
NCORES = 8
SHARD = SEQ // NCORES  # 1024
NT = SHARD // 128  # 8 key tiles per shard

_PROG_CACHE = {}

# packA column map ([128, 12] f32)
PA_SC = 0      # unused (sc is bf16, separate)
PA_WT1 = 1     # Wt1^T in rows 0:32
PA_BT1 = 2     # bt1 in rows 0:32
PA_BT2 = 3     # bt2 column
PA_BA1 = 4     # ba1 column
PA_WA2 = 5     # Wa2 column
PA_DG = 6      # dg^T columns 6:8
PA_TS = 8      # timestamps broadcast rows 0:32, cols 8:10
PA_BA2 = 10    # ba2 broadcast rows 0:2
PA_W = 12


def _build(use_collective: bool):
    import concourse.bacc as bacc
    import concourse.bass as bass
    import concourse.tile as tile
    from concourse import mybir
    from concourse.tile_rust import add_dep_helper

    AF = mybir.ActivationFunctionType
    ALU = mybir.AluOpType
    f32 = mybir.dt.float32
    f32r = mybir.dt.float32r
    bf16 = mybir.dt.bfloat16

    nc = bacc.Bacc(
        "TRN2",
        target_bir_lowering=False,
        debug=False,
        num_devices=NCORES,
    )

    def din(name, shape, dt=f32):
        return nc.dram_tensor(name, list(shape), dt, kind="ExternalInput").ap()

    q_rows = SHARD if use_collective else SEQ
    qs = din("qs", (q_rows, H), bf16)
    ks = din("ks", (SHARD, H), bf16)
    packA = din("packA", (128, PA_W))
    packB = din("packB", (128, 256))       # Wt2 (rows 0:32) | Wa1m
    packC = din("packC", (K, 1152))        # bg2 | halves
    Wgt = din("Wgt", (F, H), bf16)
    Wa1q = din("Wa1q", (128, 8, 128))      # host pre-interleaved, x(1/SEQ)
    out = nc.dram_tensor("out", [SEQ, SHARD], bf16, kind="ExternalOutput").ap()

    def r(ap):
        return ap.bitcast(f32r)

    with tile.TileContext(nc) as tc:
        with (
            tc.tile_pool(name="consts", bufs=1) as cp,
            tc.tile_pool(name="work", bufs=1) as wp,
            tc.tile_pool(name="qstream", bufs=8) as qp,
            tc.tile_pool(name="scratch", bufs=3) as sp,
            tc.tile_pool(name="psum_small", bufs=1, space="PSUM") as pps,
            tc.tile_pool(name="psum_q", bufs=1, space="PSUM") as ppq,
            tc.tile_pool(name="psum_out", bufs=1, space="PSUM") as ppo,
            tc.tile_pool(name="dram", bufs=1, space="DRAM") as dp,
        ):
            # ---- packed constants on the scalar ring --------------------
            pA = cp.tile([128, PA_W], f32)
            nc.scalar.dma_start(pA, packA)
            pB = cp.tile([128, 256], f32)
            nc.scalar.dma_start(pB, packB)
            pC = cp.tile([K, 1152], f32)
            nc.scalar.dma_start(pC, packC)
            Wg_sb = cp.tile([F, H], bf16)
            nc.scalar.dma_start(Wg_sb, Wgt)
            # ones column for the (unscaled) qsum partition-reduce; the
            # 1/SEQ mean scale is folded into Wa1q host-side
            ones_b = cp.tile([128, 1], bf16)
            nc.vector.memset(ones_b, 1.0)

            # ---- query shard DMAs head the sync ring --------------------
            nq = q_rows // 128
            qv = qs.rearrange("(t p) h -> t p h", p=128)
            qtiles = []
            q_insts = []
            for i in range(nq):
                qt = qp.tile([128, H], bf16, tag="qt")
                q_insts.append(nc.sync.dma_start(qt, qv[i]))
                qtiles.append(qt)

            # warm the ACT function tables used late in the critical path
            warm1 = cp.tile([1, 1], f32)
            nc.scalar.activation(warm1, ones_b[0:1, :], AF.Tanh)
            warm2 = cp.tile([1, 1], f32)
            nc.scalar.activation(warm2, ones_b[0:1, :], AF.Sigmoid)

            # ---- temporal MLP -> memT [F, K] (DVE + one PE matmul) ------
            h1T = wp.tile([F // 4, K], f32)
            nc.vector.tensor_scalar_mul(
                h1T, pA[0 : F // 4, PA_TS : PA_TS + 2],
                pA[0 : F // 4, PA_WT1 : PA_WT1 + 1])
            nc.vector.tensor_scalar_add(
                h1T, h1T, pA[0 : F // 4, PA_BT1 : PA_BT1 + 1])
            nc.vector.tensor_relu(h1T, h1T)
            tT_ps = pps.tile([F, K], f32, tag="small")
            nc.tensor.matmul(tT_ps, lhsT=pB[0 : F // 4, 0:128], rhs=h1T,
                             start=True, stop=True)
            memT_sb = wp.tile([F, K], f32)
            nc.vector.tensor_scalar_add(memT_sb, tT_ps,
                                        pA[:, PA_BT2 : PA_BT2 + 1])
            nc.vector.tensor_add(memT_sb, memT_sb, pA[:, PA_DG : PA_DG + 2])

            # ---- scorer mem-term: haT_ps = Wa1m^T @ memT (PSUM group
            # stays open; qmean matmuls accumulate into it post-collective)
            haT_ps = pps.tile([F, K], f32, tag="ha")
            nc.tensor.matmul(haT_ps, lhsT=pB[:, 128:256], rhs=memT_sb,
                             start=True, stop=False)

            # ---- qsum accumulates on PE as query tiles land -------------
            qsum_ps = ppq.tile([1, H], f32, tag="qsum")
            for i in range(nq):
                nc.tensor.matmul(
                    qsum_ps[:, 0:512], lhsT=ones_b, rhs=qtiles[i][:, 0:512],
                    start=(i == 0), stop=(i == nq - 1),
                )
                nc.tensor.matmul(
                    qsum_ps[:, 512:1024], lhsT=ones_b, rhs=qtiles[i][:, 512:1024],
                    start=(i == 0), stop=(i == nq - 1),
                )
            qpart_sb = wp.tile([1, H], f32)
            qpart_inst = nc.scalar.copy(qpart_sb, qsum_ps)

            # ---- gate row g_0 = mem_0 @ Wg + bg  [1, H] -----------------
            memT_b = wp.tile([F, K], bf16)
            nc.vector.tensor_copy(memT_b, memT_sb)
            g0_ps = pps.tile([1, H], f32, tag="grow")
            nc.tensor.matmul(g0_ps[:, 0:512], lhsT=memT_b[:, 0:1],
                             rhs=Wg_sb[:, 0:512], start=True, stop=True)
            nc.tensor.matmul(g0_ps[:, 512:1024], lhsT=memT_b[:, 0:1],
                             rhs=Wg_sb[:, 512:1024], start=True, stop=True)
            g0_sb = wp.tile([1, H], bf16)
            nc.vector.tensor_add(g0_sb, g0_ps, pC[0:1, 0:H])
            gb0 = wp.tile([128, H], bf16, tag="gb0")
            nc.gpsimd.partition_broadcast(gb0[:, :], g0_sb[:, :])

            # ---- gate row g_1: reuses the qsum PSUM banks (freed by the
            # qpart copy), keeping the row at partition 0 for broadcast
            g1_ps = ppq.tile([1, H], f32, tag="qsum")
            nc.tensor.matmul(g1_ps[:, 0:512], lhsT=memT_b[:, 1:2],
                             rhs=Wg_sb[:, 0:512], start=True, stop=True)
            nc.tensor.matmul(g1_ps[:, 512:1024], lhsT=memT_b[:, 1:2],
                             rhs=Wg_sb[:, 512:1024], start=True, stop=True)
            g1_sb = wp.tile([1, H], bf16)
            nc.vector.tensor_add(g1_sb, g1_ps, pC[0:1, 0:H])
            gb1 = wp.tile([128, H], bf16, tag="gb1")
            nc.gpsimd.partition_broadcast(gb1[:, :], g1_sb[:, :])
            gbs = [gb0, gb1]

            # ---- collective: AllGather of the per-core query sums -------
            if use_collective:
                cc_in = dp.tile([1, H], f32)
                cc_out = dp.tile([NCORES, H], f32)
                nc.scalar.dma_start(cc_in, qpart_sb)
                nc.gpsimd.collective_compute(
                    "AllGather",
                    ALU.bypass,
                    replica_groups=[list(range(NCORES))],
                    ins=[cc_in.opt()],
                    outs=[cc_out.opt()],
                )

            # key shard: interleaved, ktiles[j][p, :] = ks[p*NT + j, :];
            # ordered behind the query-mean extraction so the query stream
            # and the collective input are never starved
            kv = ks.rearrange("(p t) h -> p t h", t=NT)
            ktiles = []
            last_ki = None
            for j in range(NT):
                kt = cp.tile([128, H], bf16, tag=f"ks{j}")
                eng = nc.scalar if j % 2 == 0 else nc.sync
                ki = eng.dma_start(kt, kv[:, j, :])
                add_dep_helper(ki.ins, qpart_inst.ins,
                               reason="key reads after query mean")
                ktiles.append(kt)
                last_ki = ki

            # Wa1q is only needed post-collective; keep its 512KB read
            # behind the key stream
            Wa1q_sb = cp.tile([128, 8, 128], f32)
            wi = nc.sync.dma_start(Wa1q_sb, Wa1q)
            add_dep_helper(wi.ins, last_ki.ins, reason="Wa1q after keys")

            if use_collective:
                # park the gather-result load on the sync ring, after keys
                qmTd8 = wp.tile([128, NCORES, 8], f32)
                nc.sync.dma_start(
                    qmTd8, cc_out[:, :].rearrange("d (p c) -> p d c", c=8)
                )

            # ---- matvec: muls on DVE, ACT accum-reductions --------------
            # rcc[p, j, k] = sum_h g_k[h] * ks[p*NT+j, h]
            rcc = wp.tile([128, NT, K], f32)
            r2_sb = wp.tile([K, SHARD], f32r)
            for j in range(NT):
                for k in range(K):
                    prod = sp.tile([128, H], bf16, tag="prod")
                    nc.vector.tensor_mul(prod, ktiles[j], gbs[k])
                    if j >= 5 and k == 1 or j >= 7:
                        # DVE finishes the reduction for 5 of 16 items to
                        # unload the ACT accumulation chain (also breaks
                        # DVE instruction fusion into pipelineable groups)
                        nc.vector.tensor_reduce(
                            rcc[:, j, k : k + 1], prod,
                            axis=mybir.AxisListType.X, op=ALU.add)
                    else:
                        junk = sp.tile([128, H], bf16, tag="junk")
                        nc.scalar.activation(
                            junk, prod, AF.Copy,
                            accum_out=rcc[:, j, k : k + 1],
                        )
                # de-interleave r incrementally on the sync ring: column
                # r2[k, p*NT+j] <- rcc[:, j, k] as each key tile finishes
                for k in range(K):
                    rsl = r2_sb[k : k + 1, :]
                    r2dst = bass.AP(tensor=rsl.tensor, offset=rsl.offset + j,
                                    ap=[list(rsl.ap[0]), [NT, 128]])
                    nc.sync.dma_start(
                        r2dst, rcc[:, j, k : k + 1].bitcast(f32r))

            # ---- post-collective: qmT, scorer, weights -----------------
            # qmT[p, c] = qmean[p*8 + c]  (interleaved reshape layout)
            qmT = wp.tile([128, 8], f32)
            if use_collective:
                # sum gathered partials over d ([p, c, d] view, reduce X)
                nc.vector.tensor_reduce(
                    qmT, qmTd8[:, :, :].rearrange("p d c -> p c d"),
                    axis=mybir.AxisListType.X, op=ALU.add,
                )
            else:
                nc.scalar.dma_start(qmT, qpart_sb[:, :])
            qmTd = wp.tile([128, 8, K], f32)
            nc.vector.tensor_copy(qmTd[:, :, 0:1], qmT[:, :].rearrange("p c -> p c ()"))
            nc.vector.tensor_copy(qmTd[:, :, 1:2], qmT[:, :].rearrange("p c -> p c ()"))
            for c in range(8):
                nc.tensor.matmul(haT_ps, lhsT=Wa1q_sb[:, c, :],
                                 rhs=qmTd[:, c, :], start=False, stop=(c == 7))
            aT_sb = wp.tile([F, K], f32)
            nc.scalar.activation(aT_sb, haT_ps, AF.Tanh,
                                 bias=pA[:, PA_BA1 : PA_BA1 + 1], scale=1.0)
            # score as a [K, 1] column: lhsT = aT (P=K), rhs = Wa2 column
            score_ps = pps.tile([K, 1], f32, tag="small")
            nc.tensor.matmul(score_ps, lhsT=aT_sb,
                             rhs=pA[:, PA_WA2 : PA_WA2 + 1],
                             start=True, stop=True)
            w_sb = wp.tile([K, 1], f32)
            nc.scalar.activation(w_sb, score_ps, AF.Sigmoid,
                                 bias=pA[0:K, PA_BA2 : PA_BA2 + 1], scale=1.0)

            # ---- final combine: out_ps[p, s] = sum_k (w_k/2) r_k[s] -----
            wrep = wp.tile([K, 128], f32r)
            nc.vector.tensor_scalar_mul(wrep, pC[:, H : H + 128], w_sb)
            out_ps = ppo.tile([128, SHARD], f32, tag="obc")
            nc.tensor.matmul(out_ps[:, 0:512], lhsT=wrep[:, :],
                             rhs=r2_sb[:, 0:512], start=True, stop=True)
            nc.tensor.matmul(out_ps[:, 512:1024], lhsT=wrep[:, :],
                             rhs=r2_sb[:, 512:1024], start=True, stop=True)
            out_sb = wp.tile([128, SHARD], bf16)
            nc.scalar.copy(out_sb[:, 0:512], out_ps[:, 0:512])
            nc.vector.tensor_copy(out_sb[:, 512:1024], out_ps[:, 512:1024])

            # ---- output: 8 big DMAs, each writes 8 row-blocks ----------
            # (stride-0 source: the same [128, SHARD] tile re-read 8x),
            # spread across the three DGE rings.
            src = out_sb[:, :]
            src8 = bass.AP(
                tensor=src.tensor, offset=src.offset,
                ap=[list(src.ap[0]), [0, 8], list(src.ap[1])],
            )
            outv = out.rearrange("(g b p) n -> g p b n", p=128, b=8)
            rings = [nc.sync, nc.scalar, nc.gpsimd, nc.sync,
                     nc.scalar, nc.gpsimd, nc.sync, nc.scalar]
            for gidx in range(SEQ // 128 // 8):
                rings[gidx].dma_start(outv[gidx], src8)

    nc.compile()
    return nc


def _get_prog(use_collective: bool):
    key = bool(use_collective)
    if key not in _PROG_CACHE:
        _PROG_CACHE[key] = _build(key)
    return _PROG_CACHE[key]


def _make_in_maps(inputs, use_collective: bool):
    import ml_dtypes

    bf16 = ml_dtypes.bfloat16
    f32 = np.float32
    q = np.asarray(np.asarray(inputs["query"], f32)[0], bf16)  # [S,H]
    k = np.asarray(np.asarray(inputs["key"], f32)[0], bf16)  # [S,H]

    def a(x):
        return np.ascontiguousarray(np.asarray(x, f32))

    dgT = a(inputs["dg_features"]).T          # [F, K]
    ts = a(inputs["timestamps"])              # [K]
    Wt1 = a(inputs["Wt1"])                    # [1, 32]
    bt1 = a(inputs["bt1"])                    # [32]
    Wt2 = a(inputs["Wt2"])                    # [32, 128]
    bt2 = a(inputs["bt2"])                    # [128]
    Wa1 = a(inputs["Wa1"])                    # [1152, 128]
    ba1 = a(inputs["ba1"])                    # [128]
    Wa2 = a(inputs["Wa2"])                    # [128, 1]
    ba2 = a(inputs["ba2"])                    # [1]
    Wg = a(inputs["Wg"])                      # [128, 1024]
    bg = a(inputs["bg"])                      # [1024]

    packA = np.zeros((128, PA_W), f32)
    packA[0:32, PA_WT1] = Wt1[0]
    packA[0:32, PA_BT1] = bt1
    packA[:, PA_BT2] = bt2
    packA[:, PA_BA1] = ba1
    packA[:, PA_WA2] = Wa2[:, 0]
    packA[:, PA_DG : PA_DG + K] = dgT
    packA[0:32, PA_TS : PA_TS + K] = ts[None, :]
    packA[0:K, PA_BA2] = ba2[0]

    packB = np.zeros((128, 256), f32)
    packB[0:32, 0:128] = Wt2
    packB[:, 128:256] = Wa1[0:F, :]

    packC = np.zeros((K, 1152), f32)
    packC[:, 0:H] = bg[None, :]
    packC[:, H : H + 128] = 0.5

    # qmean rows of Wa1 re-paired to the interleaved qmT layout:
    # Wa1q[i, c, :] = Wa1[F + 8i + c, :]
    Wa1q = np.ascontiguousarray(
        Wa1[F : F + H, :].reshape(128, 8, 128) * (1.0 / SEQ)
    )

    common = {
        "packA": packA,
        "packB": packB,
        "packC": packC,
        "Wgt": np.asarray(Wg, bf16),
        "Wa1q": Wa1q,
    }
    in_maps = []
    for d in range(NCORES):
        m = dict(common)
        m["ks"] = np.ascontiguousarray(k[d * SHARD : (d + 1) * SHARD])
        if use_collective:
            m["qs"] = np.ascontiguousarray(q[d * SHARD : (d + 1) * SHARD])
        else:
            m["qs"] = q
        in_maps.append(m)
    return in_maps


def _run(inputs, use_collective: bool, trace: bool = False):
    from concourse.bass_utils import run_bass_kernel_spmd

    nc = _get_prog(use_collective)
    in_maps = _make_in_maps(inputs, use_collective)
    res = run_bass_kernel_spmd(
        nc, in_maps, core_ids=list(range(NCORES)), trace=trace
    )
    full = np.empty((1, 1, SEQ, SEQ), np.float32)
    for d in range(NCORES):
        full[0, 0, :, d * SHARD : (d + 1) * SHARD] = np.asarray(
            res.results[d]["out"], np.float32
        )
    return full, res


def kernel(**inputs) -> np.ndarray:
    use_collective = os.environ.get("CA1_NO_COLLECTIVE", "0") != "1"
    try:
        full, _ = _run(inputs, use_collective)
        return full
    except Exception:
        if not use_collective:
            raise
        # fall back to the zero-communication variant (replicated query)
        _PROG_CACHE.pop(True, None)
        full, _ = _run(inputs, False)
        return full


# revision 29
# speedup vs baseline: 1.7274x; 1.0214x over previous
"""Trainium2 Bass kernel for nn_CA1AttentionGate.

Computes, for full inputs (B=1, S=8192, H=1024, F=128, K=2):
    temporal = relu(t @ Wt1 + bt1) @ Wt2 + bt2          [K,F]
    mem      = dg_features + temporal                    [K,F]
    qmean    = query.mean(axis=1)                        [1,H]
    score_k  = tanh([mem_k ; qmean] @ Wa1 + ba1) @ Wa2 + ba2
    w_k      = sigmoid(score_k)
    g_k      = mem_k @ Wg + bg                           [K,H]
    row[s]   = (1/K) * sum_k w_k * (g_k . key[s])        [S]
    out      = broadcast(row) -> [1,1,S,S]

Sharding: sequence-parallel over the key/seq axis across 8 cores.  Each
core computes the final gate row for its 1024 key positions and writes
its dense [8192, 1024] column slab of the output.  The only cross-core
quantity is qmean: each core reduces its query shard on the PE engine
(accumulating PSUM matmuls per arriving tile) and a 4KB AllGather
completes the mean (fallback variant replicates the full query read).

Bandwidth plan (memory-bound problem):
  - query/key stream and the output slab travel as bf16 (host converts;
    error ~5e-3 rel, gate is 2e-2); weights/accumulation stay f32
  - small constants are packed host-side into three dense blocks so the
    SWDGE loads are single-descriptor-run DMAs (column scatter loads
    cost ~3us of Pool-sequencer descriptor generation each)
  - qsum accumulates on PE as query tiles land; the scorer's mem-term
    is precomputed pre-collective; post-collective only the qmean
    matmuls + tanh/score/sigmoid remain
  - final row combine sum_k w_k r_k is one K=2 PE matmul into a
    [128, 1024] PSUM broadcast tile
  - output: 8 big DMAs (stride-0 source re-read, 8 row-blocks each)
    spread across the sync/scalar/gpsimd rings
"""

import os

import numpy as np

SEQ = 8192
H = 1024
F = 128
K = 2
NCORES = 8
SHARD = SEQ // NCORES  # 1024
NT = SHARD // 128  # 8 key tiles per shard

_PROG_CACHE = {}

# packA column map ([128, 12] f32)
PA_SC = 0      # unused (sc is bf16, separate)
PA_WT1 = 1     # Wt1^T in rows 0:32
PA_BT1 = 2     # bt1 in rows 0:32
PA_BT2 = 3     # bt2 column
PA_BA1 = 4     # ba1 column
PA_WA2 = 5     # Wa2 column
PA_DG = 6      # dg^T columns 6:8
PA_TS = 8      # timestamps broadcast rows 0:32, cols 8:10
PA_BA2 = 10    # ba2 broadcast rows 0:2
PA_W = 12


def _build(use_collective: bool):
    import concourse.bacc as bacc
    import concourse.bass as bass
    import concourse.tile as tile
    from concourse import mybir
    from concourse.tile_rust import add_dep_helper

    AF = mybir.ActivationFunctionType
    ALU = mybir.AluOpType
    f32 = mybir.dt.float32
    f32r = mybir.dt.float32r
    bf16 = mybir.dt.bfloat16

    nc = bacc.Bacc(
        "TRN2",
        target_bir_lowering=False,
        debug=False,
        num_devices=NCORES,
    )

    def din(name, shape, dt=f32):
        return nc.dram_tensor(name, list(shape), dt, kind="ExternalInput").ap()

    q_rows = SHARD if use_collective else SEQ
    qs = din("qs", (q_rows, H), bf16)
    ks = din("ks", (SHARD, H), bf16)
    packA = din("packA", (128, PA_W))
    packB = din("packB", (128, 256))       # Wt2 (rows 0:32) | Wa1m
    packC = din("packC", (K, 1152))        # bg2 | halves
    Wgt = din("Wgt", (F, H), bf16)
    Wa1q = din("Wa1q", (128, 8, 128))      # host pre-interleaved, x(1/SEQ)
    out = nc.dram_tensor("out", [SEQ, SHARD], bf16, kind="ExternalOutput").ap()

    def r(ap):
        return ap.bitcast(f32r)

    with tile.TileContext(nc) as tc:
        with (
            tc.tile_pool(name="consts", bufs=1) as cp,
            tc.tile_pool(name="work", bufs=1) as wp,
            tc.tile_pool(name="qstream", bufs=8) as qp,
            tc.tile_pool(name="scratch", bufs=6) as sp,
            tc.tile_pool(name="psum_small", bufs=1, space="PSUM") as pps,
            tc.tile_pool(name="psum_q", bufs=1, space="PSUM") as ppq,
            tc.tile_pool(name="psum_out", bufs=1, space="PSUM") as ppo,
            tc.tile_pool(name="dram", bufs=1, space="DRAM") as dp,
        ):
            # ---- packed constants on the scalar ring --------------------
            pA = cp.tile([128, PA_W], f32)
            nc.scalar.dma_start(pA, packA)
            pB = cp.tile([128, 256], f32)
            nc.scalar.dma_start(pB, packB)
            pC = cp.tile([K, 1152], f32)
            nc.scalar.dma_start(pC, packC)
            Wg_sb = cp.tile([F, H], bf16)
            nc.scalar.dma_start(Wg_sb, Wgt)
            # ones column for the (unscaled) qsum partition-reduce; the
            # 1/SEQ mean scale is folded into Wa1q host-side
            ones_b = cp.tile([128, 1], bf16)
            nc.vector.memset(ones_b, 1.0)

            # ---- query shard DMAs head the sync ring --------------------
            nq = q_rows // 128
            qv = qs.rearrange("(t p) h -> t p h", p=128)
            qtiles = []
            q_insts = []
            for i in range(nq):
                qt = qp.tile([128, H], bf16, tag="qt")
                q_insts.append(nc.sync.dma_start(qt, qv[i]))
                qtiles.append(qt)

            # warm the ACT function tables used late in the critical path
            warm1 = cp.tile([1, 1], f32)
            nc.scalar.activation(warm1, ones_b[0:1, :], AF.Tanh)
            warm2 = cp.tile([1, 1], f32)
            nc.scalar.activation(warm2, ones_b[0:1, :], AF.Sigmoid)

            # ---- temporal MLP -> memT [F, K] (DVE + one PE matmul) ------
            h1T = wp.tile([F // 4, K], f32)
            nc.vector.tensor_scalar_mul(
                h1T, pA[0 : F // 4, PA_TS : PA_TS + 2],
                pA[0 : F // 4, PA_WT1 : PA_WT1 + 1])
            nc.vector.tensor_scalar_add(
                h1T, h1T, pA[0 : F // 4, PA_BT1 : PA_BT1 + 1])
            nc.vector.tensor_relu(h1T, h1T)
            tT_ps = pps.tile([F, K], f32, tag="small")
            nc.tensor.matmul(tT_ps, lhsT=pB[0 : F // 4, 0:128], rhs=h1T,
                             start=True, stop=True)
            memT_sb = wp.tile([F, K], f32)
            nc.vector.tensor_scalar_add(memT_sb, tT_ps,
                                        pA[:, PA_BT2 : PA_BT2 + 1])
            nc.vector.tensor_add(memT_sb, memT_sb, pA[:, PA_DG : PA_DG + 2])

            # ---- scorer mem-term: haT_ps = Wa1m^T @ memT (PSUM group
            # stays open; qmean matmuls accumulate into it post-collective)
            haT_ps = pps.tile([F, K], f32, tag="ha")
            nc.tensor.matmul(haT_ps, lhsT=pB[:, 128:256], rhs=memT_sb,
                             start=True, stop=False)

            # ---- qsum accumulates on PE as query tiles land -------------
            qsum_ps = ppq.tile([1, H], f32, tag="qsum")
            for i in range(nq):
                nc.tensor.matmul(
                    qsum_ps[:, 0:512], lhsT=ones_b, rhs=qtiles[i][:, 0:512],
                    start=(i == 0), stop=(i == nq - 1),
                )
                nc.tensor.matmul(
                    qsum_ps[:, 512:1024], lhsT=ones_b, rhs=qtiles[i][:, 512:1024],
                    start=(i == 0), stop=(i == nq - 1),
                )
            qpart_sb = wp.tile([1, H], f32)
            qpart_inst = nc.scalar.copy(qpart_sb, qsum_ps)

            # ---- gate row g_0 = mem_0 @ Wg + bg  [1, H] -----------------
            memT_b = wp.tile([F, K], bf16)
            nc.vector.tensor_copy(memT_b, memT_sb)
            g0_ps = pps.tile([1, H], f32, tag="grow")
            nc.tensor.matmul(g0_ps[:, 0:512], lhsT=memT_b[:, 0:1],
                             rhs=Wg_sb[:, 0:512], start=True, stop=True)
            nc.tensor.matmul(g0_ps[:, 512:1024], lhsT=memT_b[:, 0:1],
                             rhs=Wg_sb[:, 512:1024], start=True, stop=True)
            g0_sb = wp.tile([1, H], bf16)
            nc.vector.tensor_add(g0_sb, g0_ps, pC[0:1, 0:H])
            gb0 = wp.tile([128, H], bf16, tag="gb0")
            nc.gpsimd.partition_broadcast(gb0[:, :], g0_sb[:, :])

            # ---- gate row g_1: reuses the qsum PSUM banks (freed by the
            # qpart copy), keeping the row at partition 0 for broadcast
            g1_ps = ppq.tile([1, H], f32, tag="qsum")
            nc.tensor.matmul(g1_ps[:, 0:512], lhsT=memT_b[:, 1:2],
                             rhs=Wg_sb[:, 0:512], start=True, stop=True)
            nc.tensor.matmul(g1_ps[:, 512:1024], lhsT=memT_b[:, 1:2],
                             rhs=Wg_sb[:, 512:1024], start=True, stop=True)
            g1_sb = wp.tile([1, H], bf16)
            nc.vector.tensor_add(g1_sb, g1_ps, pC[0:1, 0:H])
            gb1 = wp.tile([128, H], bf16, tag="gb1")
            nc.gpsimd.partition_broadcast(gb1[:, :], g1_sb[:, :])
            gbs = [gb0, gb1]

            # ---- collective: AllGather of the per-core query sums -------
            if use_collective:
                cc_in = dp.tile([1, H], f32)
                cc_out = dp.tile([NCORES, H], f32)
                nc.scalar.dma_start(cc_in, qpart_sb)
                nc.gpsimd.collective_compute(
                    "AllGather",
                    ALU.bypass,
                    replica_groups=[list(range(NCORES))],
                    ins=[cc_in.opt()],
                    outs=[cc_out.opt()],
                )

            # key shard: interleaved, ktiles[j][p, :] = ks[p*NT + j, :];
            # ordered behind the query-mean extraction so the query stream
            # and the collective input are never starved
            kv = ks.rearrange("(p t) h -> p t h", t=NT)
            ktiles = []
            last_ki = None
            for j in range(NT):
                kt = cp.tile([128, H], bf16, tag=f"ks{j}")
                eng = nc.scalar if j % 2 == 0 else nc.sync
                ki = eng.dma_start(kt, kv[:, j, :])
                add_dep_helper(ki.ins, qpart_inst.ins,
                               reason="key reads after query mean")
                ktiles.append(kt)
                last_ki = ki

            # Wa1q is only needed post-collective; keep its 512KB read
            # behind the key stream
            Wa1q_sb = cp.tile([128, 8, 128], f32)
            wi = nc.sync.dma_start(Wa1q_sb, Wa1q)
            add_dep_helper(wi.ins, last_ki.ins, reason="Wa1q after keys")

            if use_collective:
                # park the gather-result load on the sync ring, after keys
                qmTd8 = wp.tile([128, NCORES, 8], f32)
                nc.sync.dma_start(
                    qmTd8, cc_out[:, :].rearrange("d (p c) -> p d c", c=8)
                )

            # ---- matvec: muls on DVE, ACT accum-reductions --------------
            # rcc[p, j, k] = sum_h g_k[h] * ks[p*NT+j, h]
            rcc = wp.tile([128, NT, K], f32)
            r2_sb = wp.tile([K, SHARD], f32r)
            for j in range(NT):
                for k in range(K):
                    prod = sp.tile([128, H], bf16, tag="prod")
                    nc.vector.tensor_mul(prod, ktiles[j], gbs[k])
                    if j >= 5 and k == 1 or j >= 7:
                        # DVE finishes the reduction for 5 of 16 items to
                        # unload the ACT accumulation chain (also breaks
                        # DVE instruction fusion into pipelineable groups)
                        nc.vector.tensor_reduce(
                            rcc[:, j, k : k + 1], prod,
                            axis=mybir.AxisListType.X, op=ALU.add)
                    else:
                        junk = sp.tile([128, H], bf16, tag="junk")
                        nc.scalar.activation(
                            junk, prod, AF.Copy,
                            accum_out=rcc[:, j, k : k + 1],
                        )
                # de-interleave r incrementally on the sync ring: column
                # r2[k, p*NT+j] <- rcc[:, j, k] as each key tile finishes
                for k in range(K):
                    rsl = r2_sb[k : k + 1, :]
                    r2dst = bass.AP(tensor=rsl.tensor, offset=rsl.offset + j,
                                    ap=[list(rsl.ap[0]), [NT, 128]])
                    nc.sync.dma_start(
                        r2dst, rcc[:, j, k : k + 1].bitcast(f32r))

            # ---- post-collective: qmT, scorer, weights -----------------
            # qmT[p, c] = qmean[p*8 + c]  (interleaved reshape layout)
            qmT = wp.tile([128, 8], f32)
            if use_collective:
                # sum gathered partials over d ([p, c, d] view, reduce X)
                nc.vector.tensor_reduce(
                    qmT, qmTd8[:, :, :].rearrange("p d c -> p c d"),
                    axis=mybir.AxisListType.X, op=ALU.add,
                )
            else:
                nc.scalar.dma_start(qmT, qpart_sb[:, :])
            qmTd = wp.tile([128, 8, K], f32)
            nc.vector.tensor_copy(qmTd[:, :, 0:1], qmT[:, :].rearrange("p c -> p c ()"))
            nc.vector.tensor_copy(qmTd[:, :, 1:2], qmT[:, :].rearrange("p c -> p c ()"))
            for c in range(8):
                nc.tensor.matmul(haT_ps, lhsT=Wa1q_sb[:, c, :],
                                 rhs=qmTd[:, c, :], start=False, stop=(c == 7))
            aT_sb = wp.tile([F, K], f32)
            nc.scalar.activation(aT_sb, haT_ps, AF.Tanh,
                                 bias=pA[:, PA_BA1 : PA_BA1 + 1], scale=1.0)
            # score as a [K, 1] column: lhsT = aT (P=K), rhs = Wa2 column
            score_ps = pps.tile([K, 1], f32, tag="small")
            nc.tensor.matmul(score_ps, lhsT=aT_sb,
                             rhs=pA[:, PA_WA2 : PA_WA2 + 1],
                             start=True, stop=True)
            w_sb = wp.tile([K, 1], f32)
            nc.scalar.activation(w_sb, score_ps, AF.Sigmoid,
                                 bias=pA[0:K, PA_BA2 : PA_BA2 + 1], scale=1.0)

            # ---- final combine: out_ps[p, s] = sum_k (w_k/2) r_k[s] -----
            wrep = wp.tile([K, 128], f32r)
            nc.vector.tensor_scalar_mul(wrep, pC[:, H : H + 128], w_sb)
            out_ps = ppo.tile([128, SHARD], f32, tag="obc")
            nc.tensor.matmul(out_ps[:, 0:512], lhsT=wrep[:, :],
                             rhs=r2_sb[:, 0:512], start=True, stop=True)
            nc.tensor.matmul(out_ps[:, 512:1024], lhsT=wrep[:, :],
                             rhs=r2_sb[:, 512:1024], start=True, stop=True)
            out_sb = wp.tile([128, SHARD], bf16)
            nc.scalar.copy(out_sb[:, 0:512], out_ps[:, 0:512])
            nc.vector.tensor_copy(out_sb[:, 512:1024], out_ps[:, 512:1024])

            # ---- output: 8 big DMAs, each writes 8 row-blocks ----------
            # (stride-0 source: the same [128, SHARD] tile re-read 8x),
            # spread across the three DGE rings.
            src = out_sb[:, :]
            src8 = bass.AP(
                tensor=src.tensor, offset=src.offset,
                ap=[list(src.ap[0]), [0, 8], list(src.ap[1])],
            )
            outv = out.rearrange("(g b p) n -> g p b n", p=128, b=8)
            rings = [nc.sync, nc.scalar, nc.gpsimd, nc.sync,
                     nc.scalar, nc.gpsimd, nc.sync, nc.scalar]
            for gidx in range(SEQ // 128 // 8):
                rings[gidx].dma_start(outv[gidx], src8)

    nc.compile()
    return nc


def _get_prog(use_collective: bool):
    key = bool(use_collective)
    if key not in _PROG_CACHE:
        _PROG_CACHE[key] = _build(key)
    return _PROG_CACHE[key]


def _make_in_maps(inputs, use_collective: bool):
    import ml_dtypes

    bf16 = ml_dtypes.bfloat16
    f32 = np.float32
    q = np.asarray(np.asarray(inputs["query"], f32)[0], bf16)  # [S,H]
    k = np.asarray(np.asarray(inputs["key"], f32)[0], bf16)  # [S,H]

    def a(x):
        return np.ascontiguousarray(np.asarray(x, f32))

    dgT = a(inputs["dg_features"]).T          # [F, K]
    ts = a(inputs["timestamps"])              # [K]
    Wt1 = a(inputs["Wt1"])                    # [1, 32]
    bt1 = a(inputs["bt1"])                    # [32]
    Wt2 = a(inputs["Wt2"])                    # [32, 128]
    bt2 = a(inputs["bt2"])                    # [128]
    Wa1 = a(inputs["Wa1"])                    # [1152, 128]
    ba1 = a(inputs["ba1"])                    # [128]
    Wa2 = a(inputs["Wa2"])                    # [128, 1]
    ba2 = a(inputs["ba2"])                    # [1]
    Wg = a(inputs["Wg"])                      # [128, 1024]
    bg = a(inputs["bg"])                      # [1024]

    packA = np.zeros((128, PA_W), f32)
    packA[0:32, PA_WT1] = Wt1[0]
    packA[0:32, PA_BT1] = bt1
    packA[:, PA_BT2] = bt2
    packA[:, PA_BA1] = ba1
    packA[:, PA_WA2] = Wa2[:, 0]
    packA[:, PA_DG : PA_DG + K] = dgT
    packA[0:32, PA_TS : PA_TS + K] = ts[None, :]
    packA[0:K, PA_BA2] = ba2[0]

    packB = np.zeros((128, 256), f32)
    packB[0:32, 0:128] = Wt2
    packB[:, 128:256] = Wa1[0:F, :]

    packC = np.zeros((K, 1152), f32)
    packC[:, 0:H] = bg[None, :]
    packC[:, H : H + 128] = 0.5

    # qmean rows of Wa1 re-paired to the interleaved qmT layout:
    # Wa1q[i, c, :] = Wa1[F + 8i + c, :]
    Wa1q = np.ascontiguousarray(
        Wa1[F : F + H, :].reshape(128, 8, 128) * (1.0 / SEQ)
    )

    common = {
        "packA": packA,
        "packB": packB,
        "packC": packC,
        "Wgt": np.asarray(Wg, bf16),
        "Wa1q": Wa1q,
    }
    in_maps = []
    for d in range(NCORES):
        m = dict(common)
        m["ks"] = np.ascontiguousarray(k[d * SHARD : (d + 1) * SHARD])
        if use_collective:
            m["qs"] = np.ascontiguousarray(q[d * SHARD : (d + 1) * SHARD])
        else:
            m["qs"] = q
        in_maps.append(m)
    return in_maps


def _run(inputs, use_collective: bool, trace: bool = False):
    from concourse.bass_utils import run_bass_kernel_spmd

    nc = _get_prog(use_collective)
    in_maps = _make_in_maps(inputs, use_collective)
    res = run_bass_kernel_spmd(
        nc, in_maps, core_ids=list(range(NCORES)), trace=trace
    )
    full = np.empty((1, 1, SEQ, SEQ), np.float32)
    for d in range(NCORES):
        full[0, 0, :, d * SHARD : (d + 1) * SHARD] = np.asarray(
            res.results[d]["out"], np.float32
        )
    return full, res


def kernel(**inputs) -> np.ndarray:
    use_collective = os.environ.get("CA1_NO_COLLECTIVE", "0") != "1"
    try:
        full, _ = _run(inputs, use_collective)
        return full
    except Exception:
        if not use_collective:
            raise
        # fall back to the zero-communication variant (replicated query)
        _PROG_CACHE.pop(True, None)
        full, _ = _run(inputs, False)
        return full
